# revision 9
# baseline (speedup 1.0000x reference)
"""Multi-head self-attention on 8 Trainium2 NeuronCores.

Problem: x(2,2048,1024), 16 heads of 64, fp32 reference. Sharding: batch (2) x
head-groups (4 groups of 4 heads). Each core computes Q/K/V projections for its
256 head-dims, attention for its 4 heads, and a partial out-projection (its 256
rows of Wo). Host sums the 4 group-partials per batch (the tensor-parallel
all-reduce) and adds bo.

Kernel layout (per core), v7 — PE-efficiency-first (HW-probed):
  Measured on HW: ScalarE activations run ~2 elem/lane/cycle for fp16 out
  (~580ns per [128,1024] exp), so ScalarE has large slack; the PE is the
  critical engine.  A slot-granular [scores -> exp -> attnV] interleave
  (probe C) streams the PE at ~98% (row-group score pairs overlap, weight
  loads hidden), vs +78% cost for the tick-granular pipeline.
  Structure: 8 passes (pair, sc-512-chunk); per pass 16 t-tiles:
    scoresT[t,s] for both heads via two concurrent K=64 row-group matmuls
    into one [128,1024] PSUM slot (ring bufs=2); exp(x/8) PSUM->SBUF fp16 on
    ScalarE; attnV (V_aug^T @ attnT, M=65, row 64 = denominator) lags one
    t-tile, accumulating into the pass's ctx PSUM slot (2 banks).
  Projections: QK pair-0 upfront; V interleaved into pass 0 (V t-tile tt
  lands right after scores tt, one slot ahead of its attnV use); QK pair-1
  interleaved into passes 1-2.  All QK evacuations (PSUM->fp16 + bias) ride
  ScalarE (Identity w/ per-partition bias AP) — fast-queue slot release.
  Normalize per pass: DVE reciprocal (fp16), GpSimd broadcast, DVE mul.
  out partial = ctx @ Wo per 512-wide n-chunk in a separate 2x1-bank PSUM
  ring, evacuated fp16 by ScalarE copies, DMA out fp16 (host sums in fp32),
  drained one chunk per even t-tile through the following passes.
  PSUM: scores 2x2 + ctx 1x2 + out 2x1 = 8 banks.
"""

import sys

sys.path.insert(0, "/opt/trn_rl_repo")

import numpy as np

import concourse.bacc as bacc
import concourse.mybir as mybir
import concourse.tile as tile
from concourse import bass_utils

N_CORES = 8
B, S, D = 2, 2048, 1024
H_LOC = 4          # heads per core
DH = 64            # head dim
DG = H_LOC * DH    # 256 group dims per core
KC = D // 128      # 8 contraction chunks over D
ST = S // 128      # 16 s/t tiles
SC = S // 512      # 4 512-wide s chunks
MT = DG // 128     # 2 m-tiles of group dims

F32 = mybir.dt.float32
F16 = mybir.dt.float16
AF = mybir.ActivationFunctionType

# which QK pair-1 s-chunks are emitted (interleaved) during which pass
QK1_SCHED = {0: [0, 1], 1: [2, 3]}


def _build_program(reps=1, num_devices=N_CORES):
    nc = bacc.Bacc("TRN2", target_bir_lowering=False, debug=False,
                   num_devices=num_devices)

    xT_d = nc.dram_tensor("xT", [KC, 128, S], F16, kind="ExternalInput")
    wq_d = nc.dram_tensor("wq", [KC, 128, DG], F16, kind="ExternalInput")
    wk_d = nc.dram_tensor("wk", [KC, 128, DG], F16, kind="ExternalInput")
    wv_d = nc.dram_tensor("wv", [KC, 128, DG], F16, kind="ExternalInput")
    bq_d = nc.dram_tensor("bq", [MT, 128, 1], F32, kind="ExternalInput")
    bk_d = nc.dram_tensor("bk", [MT, 128, 1], F32, kind="ExternalInput")
    bv_d = nc.dram_tensor("bv", [1, DG], F32, kind="ExternalInput")
    wo_d = nc.dram_tensor("wo", [MT, 128, D], F16, kind="ExternalInput")
    out_d = nc.dram_tensor("out", [S, D], F16, kind="ExternalOutput")

    with tile.TileContext(nc) as tc:
      for _rep in range(reps):
        with (
            tc.tile_pool(name="wpool", bufs=1) as wpool,
            tc.tile_pool(name="mpool", bufs=1) as mpool,
            tc.tile_pool(name="psum", bufs=2, space="PSUM") as pp,
            tc.tile_pool(name="cpsum", bufs=2, space="PSUM") as cp,
            tc.tile_pool(name="apool", bufs=1) as apool,
            tc.tile_pool(name="xpool", bufs=1) as xpool,
        ):
            # ---- weights / biases ----
            wq_t = wpool.tile([128, KC, DG], F16)
            wk_t = wpool.tile([128, KC, DG], F16)
            wv_t = wpool.tile([128, KC, DG], F16)
            wo_t = wpool.tile([128, MT, D], F16)
            bq_t = wpool.tile([128, MT], F32)
            bk_t = wpool.tile([128, MT], F32)
            bv_row = wpool.tile([1, DG], F32)
            bv_bc = wpool.tile([128, DG], F32)
            for m in range(MT):
                nc.sync.dma_start(wo_t[:, m, :], wo_d.ap()[m])
                nc.sync.dma_start(bq_t[:, m : m + 1], bq_d.ap()[m])
                nc.sync.dma_start(bk_t[:, m : m + 1], bk_d.ap()[m])
            nc.sync.dma_start(bv_row[:], bv_d.ap())
            nc.gpsimd.partition_broadcast(bv_bc[:], bv_row[:])

            # ---- persistent intermediates (all fp16 matmul operands) ----
            qT_t = mpool.tile([128, MT, S], F16)    # [dg_row, mt, s]
            kT_t = mpool.tile([128, MT, S], F16)
            vaug = mpool.tile([128, ST, H_LOC, DH + 1], F16)
            ctxN = mpool.tile([128, MT, S], F16)    # normalized ctx^T
            nc.gpsimd.memset(vaug[:], 1.0)

            xT_t = xpool.tile([128, KC, S], F16)
            for k in range(KC):
                nc.sync.dma_start(xT_t[:, k, :], xT_d.ap()[k])
                nc.sync.dma_start(wq_t[:, k, :], wq_d.ap()[k])
                nc.sync.dma_start(wk_t[:, k, :], wk_d.ap()[k])
                nc.sync.dma_start(wv_t[:, k, :], wv_d.ap()[k])

            def emit_qk_sc(m, sc):
                # one (m, sc) QK projection group; PSUM->fp16(+bias) on ScalarE
                sl = slice(sc * 512, sc * 512 + 512)
                ps_qk = pp.tile([128, 1024], F32, tag="ps", name="ps_qk")
                for k in range(KC):
                    nc.tensor.matmul(
                        ps_qk[:, 0:512], wq_t[:, k, m * 128 : m * 128 + 128],
                        xT_t[:, k, sl], start=(k == 0), stop=(k == KC - 1))
                    nc.tensor.matmul(
                        ps_qk[:, 512:1024], wk_t[:, k, m * 128 : m * 128 + 128],
                        xT_t[:, k, sl], start=(k == 0), stop=(k == KC - 1))
                nc.scalar.activation(qT_t[:, m, sl], ps_qk[:, 0:512],
                                     AF.Identity, bias=bq_t[:, m : m + 1])
                nc.scalar.activation(kT_t[:, m, sl], ps_qk[:, 512:1024],
                                     AF.Identity, bias=bk_t[:, m : m + 1])

            def emit_v_st(st):
                ps_v = pp.tile([128, 1024], F32, tag="ps", name="ps_v")
                for k in range(KC):
                    nc.tensor.matmul(
                        ps_v[:, 0:DG],
                        xT_t[:, k, st * 128 : st * 128 + 128],
                        wv_t[:, k, :],
                        start=(k == 0), stop=(k == KC - 1),
                    )
                nc.vector.tensor_add(
                    vaug[:, st, :, 0:DH],
                    ps_v[:, 0:DG].rearrange("p (h d) -> p h d", h=H_LOC),
                    bv_bc[:].rearrange("p (h d) -> p h d", h=H_LOC),
                )

            def emit_normalize_chunk(pair, sc, cu):
                # normalize one 512-wide s-chunk of both heads
                mt = pair
                ssl = slice(sc * 512, sc * 512 + 512)
                rs_p = apool.tile([128, 8], F16, tag="rs", bufs=2, name="rs_p")
                nc.sync.dma_start(rs_p[:, 0:4], cu[64:65, 0:512])
                nc.sync.dma_start(rs_p[:, 4:8], cu[64:65, 512:1024])
                rr_p = apool.tile([128, 8], F16, tag="rr", bufs=2, name="rr_p")
                with nc.allow_low_precision(
                        reason="1/denom in fp16: denom in [50,4k], "
                               "rel err ~1e-3 vs 2e-2 budget"):
                    nc.vector.reciprocal(rr_p[:], rs_p[:])
                for i in range(2):
                    r_row = apool.tile([1, 512], F16, tag="rrow", bufs=2,
                                       name="r_row")
                    nc.sync.dma_start(r_row[:], rr_p[:, 4 * i : 4 * i + 4])
                    r_bc = apool.tile([64, 512], F16, tag="rbc", bufs=2,
                                      name="r_bc")
                    nc.gpsimd.partition_broadcast(r_bc[:], r_row[:])
                    if i == 0:
                        nc.vector.tensor_mul(
                            ctxN[0:64, mt, ssl], cu[0:64, 0:512], r_bc[:])
                    else:
                        csh = apool.tile([64, 512], F16, tag="csh", bufs=2,
                                         name="csh")
                        nc.vector.tensor_mul(
                            csh[:], cu[0:64, 512:1024], r_bc[:])
                        nc.sync.dma_start(ctxN[64:128, mt, ssl], csh[:])

            # out-projection drains as interleaved 512-wide n-chunks through
            # later passes; own 1-bank PSUM ring, ScalarE copy evacuation
            out_state = {}

            def emit_out_chunk(st_o, n):
                nsl = slice(n * 512, n * 512 + 512)
                ps_o = pp.tile([128, 1024], F32, tag="ps", name="ps_o")
                for m in range(MT):
                    nc.tensor.matmul(
                        ps_o[:, 0:512],
                        ctxN[:, m, st_o * 128 : st_o * 128 + 128],
                        wo_t[:, m, nsl],
                        start=(m == 0), stop=(m == MT - 1),
                    )
                if n == 0:
                    out_state[st_o] = apool.tile([128, 1024], F16, tag="ot",
                                                 bufs=3, name="o_t")
                o_t = out_state[st_o]
                nc.scalar.copy(o_t[:, nsl], ps_o[:, 0:512])
                if n == 1:
                    nc.sync.dma_start(
                        out_d.ap()[st_o * 128 : st_o * 128 + 128, :], o_t[:])
                    del out_state[st_o]

            # ---- QK pair-0 projection upfront ----
            for sc in range(SC):
                emit_qk_sc(0, sc)

            # ---- 4 passes (pair, s-half): probe-C slot-granular interleave
            # per t-tile: two score slot-pairs (s-chunks c0,c1) -> exps ->
            # four attnV matmuls (lag 1 t-tile) into the two ctx slots
            passes = [(0, 0), (1, 0), (0, 1), (1, 1)]
            pending_out = []
            for pi, (pair, sh) in enumerate(passes):
                ps_cs = [cp.tile([128, 1024], F32, tag="ctx",
                                 name=f"ps_ctx{c}") for c in range(2)]
                qk1_fill = list(QK1_SCHED.get(pi, []))
                ats = {}

                def emit_attn_v(tt):
                    for c in range(2):
                        for i, h in enumerate((2 * pair, 2 * pair + 1)):
                            csl = slice(i * 512, i * 512 + 512)
                            nc.tensor.matmul(
                                ps_cs[c][0:65, csl],
                                vaug[:, tt, h, :],
                                ats[tt][c][:, csl],
                                start=(tt == 0), stop=(tt == ST - 1),
                            )

                for tt in range(ST):
                    tsl = slice(tt * 128, tt * 128 + 128)
                    pair_ats = []
                    for c in range(2):
                        ssl = slice((2 * sh + c) * 512,
                                    (2 * sh + c) * 512 + 512)
                        ps_s = pp.tile([128, 1024], F32, tag="ps",
                                       name="ps_s")
                        # two K=64 matmuls in disjoint PE row groups (overlap)
                        nc.tensor.matmul(
                            ps_s[:, 0:512],
                            kT_t[0:64, pair, tsl], qT_t[0:64, pair, ssl])
                        nc.tensor.matmul(
                            ps_s[:, 512:1024],
                            kT_t[64:128, pair, tsl], qT_t[64:128, pair, ssl])
                        at = apool.tile([128, 1024], F16, tag="attnT",
                                        bufs=8, name="at")
                        nc.scalar.activation(at[:], ps_s[:], AF.Exp,
                                             scale=0.125)
                        pair_ats.append(at)
                    ats[tt] = pair_ats
                    if pi == 0:
                        # V projection rides pass 0, one slot ahead of its
                        # attnV consumer
                        emit_v_st(tt)
                    if tt > 0:
                        emit_attn_v(tt - 1)
                        del ats[tt - 1]
                    if pending_out:
                        emit_out_chunk(*pending_out.pop(0))
                    if tt % 8 == 3 and qk1_fill:
                        emit_qk_sc(1, qk1_fill.pop(0))
                emit_attn_v(ST - 1)
                del ats[ST - 1]
                while qk1_fill:
                    emit_qk_sc(1, qk1_fill.pop(0))

                for c in range(2):
                    cu = apool.tile([65, 1024], F16, tag="cu", bufs=2,
                                    name="cu")
                    nc.vector.tensor_copy(cu[:], ps_cs[c][0:65, :])
                    emit_normalize_chunk(pair, 2 * sh + c, cu)
                if pair == 1:
                    for st_o in range(sh * 8, sh * 8 + 8):
                        pending_out.append((st_o, 0))
                        pending_out.append((st_o, 1))
            while pending_out:
                emit_out_chunk(*pending_out.pop(0))

    nc.compile()
    return nc


_CACHE = {}


def _get_program():
    if "nc" not in _CACHE:
        _CACHE["nc"] = _build_program()
    return _CACHE["nc"]


def _shard_inputs(x, Wq, bq, Wk, bk, Wv, bv, Wo):
    xT16 = [
        np.ascontiguousarray(x[b].T).astype(np.float16).reshape(KC, 128, S)
        for b in range(B)
    ]
    in_maps = []
    for c in range(N_CORES):
        b, g = c // 4, c % 4
        gs = slice(g * DG, g * DG + DG)
        in_maps.append({
            "xT": xT16[b],
            "wq": np.ascontiguousarray(Wq[:, gs]).astype(np.float16).reshape(KC, 128, DG),
            "wk": np.ascontiguousarray(Wk[:, gs]).astype(np.float16).reshape(KC, 128, DG),
            "wv": np.ascontiguousarray(Wv[:, gs]).astype(np.float16).reshape(KC, 128, DG),
            "bq": np.ascontiguousarray(bq[gs]).astype(np.float32).reshape(MT, 128, 1),
            "bk": np.ascontiguousarray(bk[gs]).astype(np.float32).reshape(MT, 128, 1),
            "bv": np.ascontiguousarray(bv[gs]).astype(np.float32).reshape(1, DG),
            "wo": np.ascontiguousarray(Wo[gs, :]).astype(np.float16).reshape(MT, 128, D),
        })
    return in_maps


def kernel(x, Wq, bq, Wk, bk, Wv, bv, Wo, bo, _trace=False, _trace_kwargs=None):
    x = np.asarray(x, dtype=np.float32)
    Wq, bq = np.asarray(Wq, np.float32), np.asarray(bq, np.float32)
    Wk, bk = np.asarray(Wk, np.float32), np.asarray(bk, np.float32)
    Wv, bv = np.asarray(Wv, np.float32), np.asarray(bv, np.float32)
    Wo, bo = np.asarray(Wo, np.float32), np.asarray(bo, np.float32)

    nc = _get_program()
    in_maps = _shard_inputs(x, Wq, bq, Wk, bk, Wv, bv, Wo)
    kwargs = {}
    if _trace:
        kwargs["trace"] = True
        kwargs.update(_trace_kwargs or {})
    res = bass_utils.run_bass_kernel_spmd(
        nc, in_maps, core_ids=list(range(N_CORES)), **kwargs)

    out = np.zeros((B, S, D), dtype=np.float32)
    for c in range(N_CORES):
        out[c // 4] += res.results[c]["out"].astype(np.float32)
    out += bo
    if _trace:
        kernel.last_result = res
    return out
